# revision 28
# baseline (speedup 1.0000x reference)
"""Trainium2 Bass kernel for nn_Connection_75411035783724 (Mamba2 block + MLP head).

Sharding: tensor-parallel over the 32 Mamba2 heads across 8 cores (4 heads each).
Each core computes the in_proj column-slice it needs (its x-channels + B + dt),
the depthwise causal conv (as 4 accumulating diagonal matmuls on the PE),
and a chunked-SSD scan (chunk 256): per-chunk states via PE matmuls over
transposed activations, inter-chunk recurrence via a single tensor_tensor_scan.
Only the 32 frame-start tokens are ever projected to the output (the reference
discards all other rows), so the gated RMSNorm + out_proj + MLP run on 32 rows
only (launch 2, column-sharded MLP2).

Key numeric choices: bf16 for matmul operands and big intermediates, fp32 PSUM
accumulation and fp32 for the dt/decay pipeline.
"""
import os
import sys
import numpy as np
import ml_dtypes

sys.path.insert(0, "/opt/trn_rl_repo")

import concourse.bass as bass
import concourse.tile as tile
import concourse.mybir as mybir
from concourse import bacc
from concourse import bass_utils

F32 = mybir.dt.float32
BF16 = mybir.dt.bfloat16
AF = mybir.ActivationFunctionType
OP = mybir.AluOpType
BF = ml_dtypes.bfloat16

# Model dims
D_MODEL = 1024
HIDDEN = 4096
D_STATE = 128       # n
D_CONV = 4
D_INNER = 2048
HEADDIM = 64        # p
NHEADS = 32
CONV_DIM = D_INNER + 2 * D_STATE            # 2304
D_IN_PROJ = 2 * D_INNER + 2 * D_STATE + NHEADS  # 4384
L = 8192            # tokens
NPOS = 32           # output positions (first token of each frame)
POS_STRIDE = 256
NCORES = 8
HPC = 4             # heads per core
Q = 256             # chunk length
NCHUNK = L // Q     # 32
KT = D_MODEL // 128  # 8 K-tiles
NG = 16             # token groups of 512
GSZ = 512
# in_proj col slice per core: [x 256 | B 128]; dt handled via its own tensor
NCOL = 256 + 128  # 384
MT_SPEC = [(0, 128), (128, 128), (256, 128)]  # (col0, width)


def _bf(x):
    return np.ascontiguousarray(np.asarray(x, dtype=np.float32)).astype(BF)


def _f32(x):
    return np.ascontiguousarray(np.asarray(x, dtype=np.float32))


# ----------------------------------------------------------------------------
# Launch 1 program: in_proj + conv + scan -> gated y at the 32 positions
# ----------------------------------------------------------------------------
_L1 = None


def build_l1():
    global _L1
    if _L1 is not None:
        return _L1
    nc = bacc.Bacc("TRN2", target_bir_lowering=False, debug=False,
                   num_devices=NCORES)

    def din(name, shape, dt):
        return nc.dram_tensor(name, shape, dt, kind="ExternalInput").ap()

    xT = din("xT", (D_MODEL, L), BF16)
    xTpos = din("xTpos", (D_MODEL, NPOS), BF16)
    xTwin = din("xTwin", (D_MODEL, NPOS * D_CONV), BF16)
    w_in = din("w_in", (KT, 128, NCOL), BF16)
    w_dt = din("w_dt", (KT, 128, HPC), BF16)
    w_c = din("w_c", (KT, 128, 128), BF16)
    w_z = din("w_z", (KT, 128, 256), BF16)
    diag_w = din("diag_w", (3, D_CONV, 128, 128), BF16)
    cw_c = din("cw_c", (128, D_CONV), F32)
    conv_b = din("conv_b", (128, 3), F32)
    conv_b_c = din("conv_b_c", (128, 1), F32)
    dtb128 = din("dtb128", (128, 1), F32)
    A128 = din("A128", (128, 1), F32)
    D128 = din("D128", (128, 1), F32)
    w_oT = din("w_oT", (2, 128, D_MODEL), BF16)
    ssq_out = nc.dram_tensor("ssq", (1, 2, NPOS), F32,
                             kind="ExternalOutput").ap()
    yp_out = nc.dram_tensor("yp", (128, 8, NPOS), F32,
                            kind="ExternalOutput").ap()

    BSZ = 2 * GSZ          # 1024-token batches
    NB = L // BSZ          # 8
    CPB = BSZ // Q         # 4 chunks per batch

    with tile.TileContext(nc) as tc:
        import contextlib
        with contextlib.ExitStack() as ctx:
            sb = ctx.enter_context(tc.tile_pool(name="sb", bufs=1))
            ring = ctx.enter_context(tc.tile_pool(name="ring", bufs=1))
            dsc = ctx.enter_context(tc.tile_pool(name="dsc", bufs=1, space="DRAM"))
            psA = ctx.enter_context(tc.tile_pool(name="psA", bufs=1, space="PSUM"))

            # ---- position-window inputs first: the pre-loop C32/z32 block
            # depends only on these, so the PE can start early
            xtp_sb = sb.tile([128, KT, NPOS], BF16)
            nc.sync.dma_start(out=xtp_sb, in_=xTpos.rearrange("(k p) t -> p k t", p=128))
            xtw_sb = sb.tile([128, KT, NPOS * D_CONV], BF16)
            nc.sync.dma_start(out=xtw_sb, in_=xTwin.rearrange("(k p) t -> p k t", p=128))
            w_c_sb = sb.tile([128, KT, 128], BF16)
            nc.sync.dma_start(out=w_c_sb, in_=w_c.rearrange("k p c -> p k c"))
            w_z_sb = sb.tile([128, KT, 256], BF16)
            nc.sync.dma_start(out=w_z_sb, in_=w_z.rearrange("k p c -> p k c"))
            cw_sb = sb.tile([128, D_CONV], F32)
            nc.sync.dma_start(out=cw_sb, in_=cw_c)
            cbc_sb = sb.tile([128, 1], F32)
            nc.sync.dma_start(out=cbc_sb, in_=conv_b_c)
            # ---- main-loop weights
            w_in_sb = sb.tile([128, KT, NCOL], BF16)
            nc.sync.dma_start(out=w_in_sb, in_=w_in.rearrange("k p c -> p k c"))
            w_dt_sb = sb.tile([128, KT, HPC], BF16)
            nc.sync.dma_start(out=w_dt_sb, in_=w_dt.rearrange("k p c -> p k c"))
            diag_sb = sb.tile([128, 3, D_CONV, 128], BF16)
            nc.sync.dma_start(out=diag_sb, in_=diag_w.rearrange("c j a b -> a c j b"))
            cb_sb = sb.tile([128, 3], F32)
            nc.sync.dma_start(out=cb_sb, in_=conv_b)
            dtb_sb = sb.tile([128, 1], F32)
            nc.sync.dma_start(out=dtb_sb, in_=dtb128)
            A_sb = sb.tile([128, 1], F32)
            nc.sync.dma_start(out=A_sb, in_=A128)
            D_sb = sb.tile([128, 1], F32)
            nc.sync.dma_start(out=D_sb, in_=D128)
            woT_sb = sb.tile([128, 2, D_MODEL], BF16)
            nc.sync.dma_start(out=woT_sb, in_=w_oT.rearrange("k p c -> p k c"))

            # ---- persistent big buffers
            xbcc = sb.tile([128, 3, L], BF16, tag="big1")  # conv+silu [x0|x1|B]
            XT = sb.tile([128, L // 128, 256], BF16, tag="big2")  # transposed x*w
            BT = sb.tile([128, L // 128, 128], BF16, tag="bt")    # transposed B
            S_all = sb.tile([128, HPC * HEADDIM * NCHUNK], BF16)  # (h,p,c)
            # dt in (c,h)-partition layout: partition p = c*HPC + h, free = token
            # within chunk. All decay-pipe math runs on [128, 256] tiles.
            dt_rs = sb.tile([128, Q], F32)
            exp_t = sb.tile([128, Q], F32)
            dt2 = sb.tile([128, Q], F32)
            a2 = sb.tile([128, Q], F32)
            s2 = sb.tile([128, Q], F32)
            w_t = sb.tile([128, Q], F32)
            w16 = sb.tile([128, Q], BF16)
            ones_t = sb.tile([128, Q], F32)
            nc.vector.memset(ones_t, 1.0)
            x32 = sb.tile([128, 2, NPOS], F32)
            B32 = sb.tile([128, NPOS], F32)

            # ================= pre-loop: C32 / z32 from window data =========
            C32 = sb.tile([128, NPOS], F32)
            pcw = psA.tile([128, NPOS * D_CONV], F32, tag="pcv")
            for k in range(KT):
                nc.tensor.matmul(pcw, w_c_sb[:, k, :], xtw_sb[:, k, :],
                                 start=(k == 0), stop=(k == KT - 1))
            tmpc = sb.tile([128, NPOS], F32)
            for j in range(D_CONV):
                src = bass.AP(tensor=pcw.tensor, offset=pcw.offset + j,
                              ap=[list(pcw.ap[0]), [D_CONV, NPOS]])
                if j == 0:
                    nc.vector.tensor_scalar_mul(tmpc, src, cw_sb[:, 0:1])
                else:
                    nc.vector.scalar_tensor_tensor(
                        out=tmpc, in0=src, scalar=cw_sb[:, j:j + 1], in1=tmpc,
                        op0=OP.mult, op1=OP.add)
            nc.scalar.activation(out=C32, in_=tmpc, func=AF.Silu,
                                 bias=cbc_sb[:, 0:1], scale=1.0)
            C32b = sb.tile([128, NPOS], BF16)
            nc.vector.tensor_copy(out=C32b, in_=C32)
            pz = psA.tile([128, 2, NPOS], F32, tag="pcv")
            for t in range(2):
                for k in range(KT):
                    nc.tensor.matmul(pz[:, t, :],
                                     w_z_sb[:, k, t * 128:(t + 1) * 128],
                                     xtp_sb[:, k, :],
                                     start=(k == 0), stop=(k == KT - 1))
            zs = sb.tile([128, 2, NPOS], F32)
            nc.scalar.activation(out=zs, in_=pz, func=AF.Silu)

            # ================= main loop: 8 batches of 1024 tokens ==========
            # k-outer in_proj and tap-outer conv share each stationary across
            # the two 512-token groups; conv recycles the in_proj PSUM banks.
            prev_xbc = None
            for b in range(NB):
                xts = []
                for i in range(2):
                    g = 2 * b + i
                    xt_g = ring.tile([128, KT, GSZ], BF16, tag="xt", bufs=2)
                    nc.sync.dma_start(
                        out=xt_g,
                        in_=xT.rearrange("(k p) t -> p k t", p=128)
                        [:, :, g * GSZ:(g + 1) * GSZ])
                    xts.append(xt_g)
                ps = [[psA.tile([cw_, GSZ], F32, tag=f"pin{mt}", bufs=2,
                                name=f"pin{mt}_{i}")
                       for mt, (c0, cw_) in enumerate(MT_SPEC)]
                      for i in range(2)]
                for k in range(KT):
                    for mt, (c0, cw_) in enumerate(MT_SPEC):
                        for i in range(2):
                            nc.tensor.matmul(ps[i][mt],
                                             w_in_sb[:, k, c0:c0 + cw_],
                                             xts[i][:, k, :],
                                             start=(k == 0), stop=(k == KT - 1))
                # dt matmuls -> PSUM -> SBUF stage -> DMA into (c,h) partitions
                for i in range(2):
                    g = 2 * b + i
                    pdt = psA.tile([HPC, GSZ], F32, tag="pdt", bufs=1)
                    for k in range(KT):
                        nc.tensor.matmul(pdt, w_dt_sb[:, k, :], xts[i][:, k, :],
                                         start=(k == 0), stop=(k == KT - 1))
                    dtst = ring.tile([HPC, GSZ], F32, tag="dtst", bufs=1)
                    nc.vector.tensor_copy(out=dtst, in_=pdt)
                    src = bass.AP(tensor=dtst.tensor, offset=dtst.offset,
                                  ap=[[Q, 2], [GSZ, HPC], [1, Q]])
                    nc.gpsimd.dma_start(out=dt_rs[8 * g:8 * g + 8], in_=src)
                # evict in_proj to the xbc ring (3 halo cols at the front)
                xbcs = []
                for i in range(2):
                    xbc_g = ring.tile([128, 3, GSZ + 3], BF16, tag="xbc", bufs=2)
                    if prev_xbc is None:
                        nc.vector.memset(xbc_g[:, :, 0:3], 0.0)
                    else:
                        nc.gpsimd.tensor_copy(out=xbc_g[:, :, 0:3],
                                              in_=prev_xbc[:, :, GSZ:GSZ + 3])
                    nc.vector.tensor_copy(out=xbc_g[:, 0, 3:], in_=ps[i][0])
                    nc.scalar.copy(out=xbc_g[:, 1, 3:], in_=ps[i][1])
                    nc.vector.tensor_copy(out=xbc_g[:, 2, 3:], in_=ps[i][2])
                    prev_xbc = xbc_g
                    xbcs.append(xbc_g)
                # conv: tap-outer, stationary shared across the two groups;
                # PSUM banks recycled from the in_proj tags
                pcs = [[psA.tile([128, GSZ], F32, tag=f"pin{cht}", bufs=2,
                                 name=f"pcv{cht}_{i}")
                        for cht in range(3)] for i in range(2)]
                for cht in range(3):
                    for j in range(D_CONV):
                        for i in range(2):
                            nc.tensor.matmul(pcs[i][cht],
                                             diag_sb[:, cht, j, :],
                                             xbcs[i][:, cht, j:j + GSZ],
                                             start=(j == 0),
                                             stop=(j == D_CONV - 1))
                for i in range(2):
                    g = 2 * b + i
                    for cht in range(3):
                        nc.scalar.activation(
                            out=xbcc[:, cht, g * GSZ:(g + 1) * GSZ],
                            in_=pcs[i][cht], func=AF.Silu,
                            bias=cb_sb[:, cht:cht + 1], scale=1.0)
                if b % 2 == 1:
                    qsl = slice((b - 1) * BSZ, (b + 1) * BSZ)
                    nc.sync.dma_start_transpose(
                        out=BT[:, 8 * (b - 1):8 * (b + 1), :],
                        in_=xbcc[:, 2, qsl])

            # ================= global decay pipe (one Exp/Ln table era) =====
            nc.scalar.activation(out=exp_t, in_=dt_rs, func=AF.Exp,
                                 bias=dtb_sb[:, 0:1], scale=1.0)
            nc.scalar.activation(out=dt2, in_=exp_t, func=AF.Ln,
                                 bias=1.0, scale=1.0)
            nc.vector.tensor_scalar_mul(a2, dt2, A_sb[:, 0:1])
            nc.vector.tensor_tensor_scan(out=s2, data0=ones_t, data1=a2,
                                         initial=0.0, op0=OP.mult, op1=OP.add)
            dAA1 = sb.tile([128, 1], F32)
            nc.scalar.activation(out=dAA1, in_=a2[:, 0:1], func=AF.Exp)
            lam1b = sb.tile([128, 1], BF16)
            nc.scalar.activation(out=lam1b, in_=s2[:, Q - 1:Q], func=AF.Exp)
            nc.vector.memset(lam1b[0:HPC], 0.0)  # chunk 0: no incoming state
            nc.vector.tensor_scalar(w_t, s2, s2[:, Q - 1:Q], None, OP.subtract)
            nc.scalar.activation(out=w_t, in_=w_t, func=AF.Exp, scale=-1.0)
            nc.vector.tensor_mul(w_t, w_t, dt2)
            nc.vector.tensor_copy(out=w16, in_=w_t)
            w_dd = dsc.tile([HPC, L], BF16)
            dst = bass.AP(tensor=w_dd.tensor, offset=w_dd.offset,
                          ap=[[Q, NCHUNK], [L, HPC], [1, Q]])
            nc.gpsimd.dma_start(out=dst, in_=w16)

            # ============ scale + transpose + chunk-state matmuls ===========
            for b in range(NB):
                bsl = slice(b * BSZ, (b + 1) * BSZ)
                wbc2 = ring.tile([128, 2, BSZ], BF16, tag="wbc", bufs=2)
                for t in range(2):
                    src = bass.AP(tensor=w_dd.tensor,
                                  offset=w_dd.offset + (2 * t) * L + b * BSZ,
                                  ap=[[L, 2], [0, 64], [1, BSZ]])
                    nc.gpsimd.dma_start(out=wbc2[:, t, :], in_=src)
                xs2 = ring.tile([128, 2, BSZ], BF16, tag="xs", bufs=2)
                nc.vector.tensor_mul(xs2[:, 0, :], xbcc[:, 0, bsl],
                                     wbc2[:, 0, :])
                nc.gpsimd.tensor_mul(xs2[:, 1, :], xbcc[:, 1, bsl],
                                     wbc2[:, 1, :])
                nc.sync.dma_start_transpose(
                    out=XT[:, 8 * b:8 * (b + 1), 0:128], in_=xs2[:, 0, :])
                nc.scalar.dma_start_transpose(
                    out=XT[:, 8 * b:8 * (b + 1), 128:256], in_=xs2[:, 1, :])
                for cc in range(CPB):
                    c = b * CPB + cc
                    pc2 = psA.tile([128, HPC * HEADDIM], F32,
                                   tag=f"pin{cc % 3}", bufs=2)
                    for k2 in range(2):
                        T = 2 * c + k2
                        nc.tensor.matmul(pc2, BT[:, T, :], XT[:, T, :],
                                         start=(k2 == 0), stop=(k2 == 1))
                    dst = bass.AP(tensor=S_all.tensor, offset=S_all.offset + c,
                                  ap=[list(S_all.ap[0]),
                                      [HEADDIM * NCHUNK, HPC], [NCHUNK, HEADDIM]])
                    if cc % 2 == 1:
                        nc.scalar.copy(out=dst, in_=pc2.rearrange(
                            "n (h p) -> n h p", h=HPC))
                    else:
                        nc.vector.tensor_copy(out=dst, in_=pc2.rearrange(
                            "n (h p) -> n h p", h=HPC))

            # ---- position extracts
            for cht in range(2):
                s_ap = bass.AP(tensor=xbcc.tensor, offset=xbcc.offset + cht * L,
                               ap=[list(xbcc.ap[0]), [POS_STRIDE, NPOS]])
                nc.vector.tensor_copy(out=x32[:, cht, :], in_=s_ap)
            s_ap = bass.AP(tensor=xbcc.tensor, offset=xbcc.offset + 2 * L,
                           ap=[list(xbcc.ap[0]), [POS_STRIDE, NPOS]])
            nc.gpsimd.tensor_copy(out=B32, in_=s_ap)

            # ================= tail =================
            # BC row = sum_n B32*C32 -> [1, NPOS] -> (c,h) partitions
            bc_t = sb.tile([128, NPOS], F32)
            nc.vector.tensor_mul(bc_t, B32, C32)
            ones1 = sb.tile([128, 1], F32)
            nc.vector.memset(ones1, 1.0)
            pbc = psA.tile([1, NPOS], F32, tag="pcv")
            nc.tensor.matmul(pbc, ones1, bc_t, start=True, stop=True)
            bc_row = sb.tile([1, NPOS], F32)
            nc.vector.tensor_copy(out=bc_row, in_=pbc)
            bc_d = dsc.tile([1, NPOS], F32)
            nc.gpsimd.dma_start(out=bc_d, in_=bc_row)
            bc128 = sb.tile([128, 1], F32)
            nc.gpsimd.dma_start(
                out=bc128, in_=bass.AP(tensor=bc_d.tensor, offset=bc_d.offset,
                                       ap=[[1, NPOS], [0, HPC], [1, 1]]))
            # f = dt_pos*BC + D on (c,h) partitions
            f4 = sb.tile([128, 1], F32)
            nc.vector.scalar_tensor_tensor(out=f4, in0=dt2[:, 0:1],
                                           scalar=bc128[:, 0:1],
                                           in1=D_sb[:, 0:1],
                                           op0=OP.mult, op1=OP.add)

            # dAA/f4 -> [128(hh,p), 2, NPOS] broadcasts via DRAM
            small_d = dsc.tile([2, 128], F32)
            nc.gpsimd.dma_start(
                out=bass.AP(tensor=small_d.tensor, offset=small_d.offset,
                            ap=[[1, 128], [1, 1]]), in_=dAA1)
            nc.gpsimd.dma_start(
                out=bass.AP(tensor=small_d.tensor, offset=small_d.offset + 128,
                            ap=[[1, 128], [1, 1]]), in_=f4)
            dA_bc = sb.tile([128, 2, NPOS], F32)
            f_bc = sb.tile([128, 2, NPOS], F32)
            for tile_, off in ((dA_bc, 0), (f_bc, 128)):
                for t in range(2):
                    for hh in range(2):
                        src = bass.AP(tensor=small_d.tensor,
                                      offset=small_d.offset + off + 2 * t + hh,
                                      ap=[[0, 64], [HPC, NPOS]])
                        nc.gpsimd.dma_start(
                            out=tile_[64 * hh:64 * (hh + 1), t, :], in_=src)

            # lambda -> lam_flat [128, (h,p,c)] via DRAM expand (multi-hop to
            # keep each DMA within the 3-dim AP balance limit); first hop
            # reorders (c,h) partitions into an (h,c) DRAM row
            lam_d = dsc.tile([HPC, NCHUNK], BF16)
            nc.gpsimd.dma_start(
                out=bass.AP(tensor=lam_d.tensor, offset=lam_d.offset,
                            ap=[[1, NCHUNK], [NCHUNK, HPC], [1, 1]]),
                in_=lam1b)
            lam_row = sb.tile([1, HPC * HEADDIM * NCHUNK], BF16)
            lam_src = bass.AP(tensor=lam_d.tensor, offset=lam_d.offset,
                              ap=[[0, 1], [NCHUNK, HPC], [0, HEADDIM],
                                  [1, NCHUNK]])
            nc.gpsimd.dma_start(
                out=lam_row.rearrange("o (h p c) -> o h p c", h=HPC,
                                      p=HEADDIM),
                in_=lam_src)
            lam_rd = dsc.tile([1, HPC * HEADDIM * NCHUNK], BF16)
            nc.gpsimd.dma_start(out=lam_rd, in_=lam_row)
            lam_flat = sb.tile([128, HPC * HEADDIM * NCHUNK], BF16, tag="bt")
            nc.gpsimd.dma_start(
                out=lam_flat,
                in_=bass.AP(tensor=lam_rd.tensor, offset=lam_rd.offset,
                            ap=[[0, 128], [1, HPC * HEADDIM * NCHUNK]]))

            # inter-chunk recurrence (Sg recycles the XT slot)
            Sg = sb.tile([128, HPC * HEADDIM * NCHUNK], BF16, tag="big2")
            nc.vector.tensor_tensor_scan(out=Sg, data0=lam_flat, data1=S_all,
                                         initial=0.0, op0=OP.mult, op1=OP.add)

            # per-position projections: y32s[(hh,p), t, pos] = C32 . Sg[:,(h,p,pos-1)]
            py = psA.tile([128, 2, NPOS], F32, tag="pcv")
            nc.vector.memset(py[:, :, 0:1], 0.0)
            first = True
            for pos in range(1, NPOS):
                for t in range(2):
                    lhs = bass.AP(
                        tensor=Sg.tensor,
                        offset=Sg.offset + (2 * t) * HEADDIM * NCHUNK + (pos - 1),
                        ap=[list(Sg.ap[0]), [HEADDIM * NCHUNK, 2], [NCHUNK, 64]])
                    nc.tensor.matmul(py[:, t, pos:pos + 1], lhs,
                                     C32b[:, pos:pos + 1],
                                     start=first, stop=(pos == NPOS - 1 and t == 1),
                                     skip_group_check=True)
                    first = False
            y32 = sb.tile([128, 2, NPOS], F32)
            nc.vector.tensor_mul(y32, py, dA_bc)
            tloc = sb.tile([128, 2, NPOS], F32)
            nc.vector.tensor_mul(tloc, x32, f_bc)
            nc.vector.tensor_add(y32, y32, tloc)
            nc.vector.tensor_mul(y32, y32, zs)
            # partial sum-of-squares of the gated y (for the global RMSNorm)
            sq32 = sb.tile([128, 2, NPOS], F32)
            nc.vector.tensor_mul(sq32, y32, y32)
            pss = psA.tile([1, 2, NPOS], F32, tag="pcv")
            for t in range(2):
                nc.tensor.matmul(pss[:, t, :], ones1, sq32[:, t, :],
                                 start=True, stop=True)
            ssq_sb = sb.tile([1, 2, NPOS], F32)
            nc.vector.tensor_copy(out=ssq_sb, in_=pss)
            nc.sync.dma_start(out=ssq_out, in_=ssq_sb)
            # partial out_proj: yp[mt*128+p, pos] = sum_ch woT[ch, m] * y[ch, pos]
            # (norm_w folded into woT host-side; the 1/rms scale is applied
            # host-side after the cross-core reduction)
            ybf = sb.tile([128, 2, NPOS], BF16)
            nc.vector.tensor_copy(out=ybf, in_=y32)
            po = psA.tile([128, 8, NPOS], F32, tag="pcv")
            for mt in range(8):
                for t in range(2):
                    nc.tensor.matmul(po[:, mt, :],
                                     woT_sb[:, t, mt * 128:(mt + 1) * 128],
                                     ybf[:, t, :], start=(t == 0), stop=(t == 1))
            yp_sb = sb.tile([128, 8, NPOS], F32)
            nc.scalar.copy(out=yp_sb, in_=po)
            nc.sync.dma_start(out=yp_out, in_=yp_sb)

    nc.compile()
    _L1 = nc
    return nc


# ----------------------------------------------------------------------------
# Launch 2 program: gated RMSNorm + out_proj + MLP on the 32 rows
# ----------------------------------------------------------------------------
_L2 = None


def build_l2():
    """Pure sharded MLP: core k owns hidden rows [512k, 512k+512) of w1/g and
    the matching 512 input columns of w2. Produces a [32, 4096] fp32 partial
    that the host sums across cores (plus b2)."""
    global _L2
    if _L2 is not None:
        return _L2
    nc = bacc.Bacc("TRN2", target_bir_lowering=False, debug=False,
                   num_devices=NCORES)

    hT = nc.dram_tensor("hT", (KT, 128, NPOS), BF16, kind="ExternalInput").ap()
    w1s = nc.dram_tensor("w1sT", (KT, 128, 512), BF16, kind="ExternalInput").ap()
    b1s = nc.dram_tensor("b1s", (128, 4), F32, kind="ExternalInput").ap()
    w2s = nc.dram_tensor("w2sT", (4, 128, HIDDEN), BF16, kind="ExternalInput").ap()
    outp = nc.dram_tensor("outp", (NPOS, HIDDEN), F32, kind="ExternalOutput").ap()

    with tile.TileContext(nc) as tc:
        import contextlib
        with contextlib.ExitStack() as ctx:
            sb = ctx.enter_context(tc.tile_pool(name="sb", bufs=1))
            psp = ctx.enter_context(tc.tile_pool(name="ps", bufs=1, space="PSUM"))

            w2_sb = sb.tile([128, 4, HIDDEN], BF16)
            nc.sync.dma_start(out=w2_sb, in_=w2s.rearrange("k p c -> p k c"))
            h_sb = sb.tile([128, KT, NPOS], BF16)
            nc.sync.dma_start(out=h_sb, in_=hT.rearrange("k p c -> p k c"))
            w1_sb = sb.tile([128, KT, 512], BF16)
            nc.sync.dma_start(out=w1_sb, in_=w1s.rearrange("k p c -> p k c"))
            b1_sb = sb.tile([128, 4], F32)
            nc.sync.dma_start(out=b1_sb, in_=b1s)

            # g = gelu(w1_slice @ h + b1_slice)  [512, 32]
            g_sb = sb.tile([128, 4, NPOS], BF16)
            for mt in range(4):
                pg = psp.tile([128, NPOS], F32, tag="pg", bufs=2)
                for k in range(KT):
                    nc.tensor.matmul(pg, w1_sb[:, k, mt * 128:(mt + 1) * 128],
                                     h_sb[:, k, :], start=(k == 0),
                                     stop=(k == KT - 1))
                nc.scalar.activation(out=g_sb[:, mt, :], in_=pg, func=AF.Gelu,
                                     bias=b1_sb[:, mt:mt + 1], scale=1.0)
            # partial out^T [32, 4096] = g^T @ w2_sliceT
            out_sb = sb.tile([NPOS, HIDDEN], F32)
            for j in range(8):
                pn = psp.tile([NPOS, 512], F32, tag="pn", bufs=4)
                for k in range(4):
                    nc.tensor.matmul(pn, g_sb[:, k, :],
                                     w2_sb[:, k, j * 512:(j + 1) * 512],
                                     start=(k == 0), stop=(k == 3))
                if j % 2 == 0:
                    nc.vector.tensor_copy(out=out_sb[:, j * 512:(j + 1) * 512],
                                          in_=pn)
                else:
                    nc.scalar.copy(out=out_sb[:, j * 512:(j + 1) * 512], in_=pn)
            nc.sync.dma_start(out=outp, in_=out_sb)

    nc.compile()
    _L2 = nc
    return nc


# ----------------------------------------------------------------------------
# Host-side prep + glue
# ----------------------------------------------------------------------------

def _prep_l1_maps(inputs):
    x = _f32(inputs["x"]).reshape(L, D_MODEL)
    xT = np.ascontiguousarray(x.T)                       # [1024, 8192]
    xT_b = _bf(xT)
    pos = np.arange(NPOS) * POS_STRIDE
    xTpos = _bf(xT[:, pos])
    # window tokens (pos, d): t*-3+d, zero-padded below 0
    win_idx = (pos[:, None] + np.arange(D_CONV)[None, :] - (D_CONV - 1)).reshape(-1)
    xTwin = np.zeros((D_MODEL, NPOS * D_CONV), np.float32)
    valid = win_idx >= 0
    xTwin[:, valid] = xT[:, win_idx[valid]]
    xTwin = _bf(xTwin)

    w_all = _f32(inputs["in_proj_w"])                    # [4384, 1024]
    conv_w = _f32(inputs["conv_w"])                      # [2304, 4]
    conv_b = _f32(inputs["conv_b"])                      # [2304]
    dt_bias = _f32(inputs["dt_bias"])                    # [32]
    A = -np.exp(_f32(inputs["A_log"]))                   # [32]
    Dp = _f32(inputs["D"])                               # [32]

    w_cT = _bf(w_all[D_INNER + D_INNER + D_STATE:
                     D_INNER + D_INNER + 2 * D_STATE].T.reshape(KT, 128, 128))
    cw_c = _f32(conv_w[D_INNER + D_STATE:])              # [128, 4] C channels
    conv_b_c = _f32(conv_b[D_INNER + D_STATE:]).reshape(128, 1)

    maps = []
    for k in range(NCORES):
        xs = 256 * k
        cols = np.concatenate([
            np.arange(D_INNER + xs, D_INNER + xs + 256),          # x slice
            np.arange(2 * D_INNER, 2 * D_INNER + D_STATE),        # B
        ])
        w_in = _bf(w_all[cols].T.reshape(KT, 128, NCOL))
        dt_cols = np.arange(D_IN_PROJ - NHEADS + HPC * k,
                            D_IN_PROJ - NHEADS + HPC * k + HPC)
        w_dt = _bf(w_all[dt_cols].T.reshape(KT, 128, HPC))
        w_z = _bf(w_all[xs:xs + 256].T.reshape(KT, 128, 256))
        # conv channels for this core: x slice (256) + B (128)
        ch_x = np.arange(xs, xs + 256)
        ch_B = np.arange(D_INNER, D_INNER + D_STATE)
        dw = np.zeros((3, D_CONV, 128, 128), np.float32)
        cb = np.zeros((128, 3), np.float32)
        for cht, chs in enumerate([ch_x[:128], ch_x[128:], ch_B]):
            for j in range(D_CONV):
                dw[cht, j] = np.diag(conv_w[chs, j])
            cb[:, cht] = conv_b[chs]
        heads = np.arange(HPC * k, HPC * k + HPC)
        # out_proj column slice for this core's 256 channels, norm_w folded in.
        # launch-1 channel layout: partition (hh*64+p), k-tile t -> ch 256k+t*128+...
        ch = 256 * k + np.arange(256)
        w_o = _f32(inputs["mamba_out_w"])                # [1024, 2048]
        nw = _f32(inputs["norm_w"])
        wo_cols = w_o[:, ch] * nw[ch][None, :]           # [1024, 256]
        w_oT = _bf(wo_cols.T.reshape(2, 128, D_MODEL))
        maps.append({
            "xT": xT_b, "xTpos": xTpos, "xTwin": xTwin,
            "w_in": w_in, "w_dt": w_dt, "w_c": w_cT, "w_z": w_z,
            "diag_w": _bf(dw), "cw_c": cw_c, "conv_b": cb,
            "conv_b_c": conv_b_c,
            "dtb128": np.tile(dt_bias[heads], NCHUNK).reshape(128, 1).astype(np.float32),
            "A128": np.tile(A[heads], NCHUNK).reshape(128, 1).astype(np.float32),
            "D128": np.tile(Dp[heads], NCHUNK).reshape(128, 1).astype(np.float32),
            "w_oT": w_oT,
        })
    return maps


def _prep_l2_maps(inputs, h):
    # h: [1024, 32] fp32 (normalized). Core k owns hidden rows 512k..512k+512.
    hT = _bf(h.reshape(KT, 128, NPOS))
    w1 = _f32(inputs["mlp_w1"])                          # [4096, 1024]
    b1 = _f32(inputs["mlp_b1"])                          # [4096]
    w2 = _f32(inputs["mlp_w2"])                          # [4096, 4096]
    maps = []
    for k in range(NCORES):
        rows = slice(512 * k, 512 * k + 512)
        w1sT = _bf(w1[rows].T.reshape(KT, 128, 512))
        b1s = _f32(b1[rows]).reshape(4, 128).transpose(1, 0).copy()
        w2sT = _bf(w2[:, rows].T.reshape(4, 128, HIDDEN))
        maps.append({
            "hT": hT, "w1sT": w1sT, "b1s": b1s, "w2sT": w2sT,
        })
    return maps


LAST_RESULTS = []


def kernel(**inputs) -> np.ndarray:
    trace = os.environ.get("KERNEL_TRACE", "0") == "1"
    LAST_RESULTS.clear()
    nc1 = build_l1()
    maps1 = _prep_l1_maps(inputs)
    res1 = bass_utils.run_bass_kernel_spmd(nc1, maps1, core_ids=list(range(NCORES)),
                                           trace=trace)
    LAST_RESULTS.append(res1)
    # cross-core reduction + RMSNorm scale on the host (tiny: [1024, 32])
    ssq = np.zeros((NPOS,), np.float64)
    y_pre = np.zeros((1024, NPOS), np.float64)
    for k in range(NCORES):
        s = res1.results[k]["ssq"]                       # [1, 2, 32]
        ssq += s[0, 0] + s[0, 1]
        yp = res1.results[k]["yp"]                       # [128, 8, 32]
        y_pre += yp.transpose(1, 0, 2).reshape(1024, NPOS)
    r = 1.0 / np.sqrt(ssq / D_INNER + 1e-5)              # [32]
    h = (y_pre * r[None, :]).astype(np.float32)          # [1024, 32]
    nc2 = build_l2()
    maps2 = _prep_l2_maps(inputs, h)
    res2 = bass_utils.run_bass_kernel_spmd(nc2, maps2, core_ids=list(range(NCORES)),
                                           trace=trace)
    LAST_RESULTS.append(res2)
    out = np.zeros((NPOS, HIDDEN), np.float64)
    for k in range(NCORES):
        out += res2.results[k]["outp"]                   # [32, 4096] partials
    out += _f32(inputs["mlp_b2"])[None, :]
    return out.astype(np.float32)

